# revision 1
# baseline (speedup 1.0000x reference)
"""GIN message-passing GNN on 8 Trainium2 NeuronCores (Bass/Tile).

Strategy (self-contained; shapes hardcoded for the 100k-node / 1.6M-edge /
128-dim / 10-layer / 64-graph problem):

- Nodes are partitioned into 8 contiguous ranges of 12500; each core owns the
  edges whose *destination* falls in its range.
- Each core keeps a full replica of the node features in its HBM. Per layer it
  gathers x[src] for its ~200k edges with one indirect DMA per 128-dst tile
  (edges pre-sorted by dst tile host-side, padded to a multiple of 128 with
  src=0 / dstoff=-1; pad length is the max over cores so the program is SPMD).
- The segment-sum (scatter-add) becomes a matmul: for each 128-edge chunk,
  PSUM[feat, dst] += contract_edges(gathered[edge, feat], onehot[edge, dst]),
  with the one-hot built on the vector engine by a broadcast is_equal against
  a resident iota row. Padding lanes have dstoff=-1 -> all-zero column.
- The GIN MLP runs in the transposed [feat, node] orientation so both matmuls
  chain without transposes; the per-core x^T slab stays resident in SBUF for
  the residual adds. Only the final per-tile result is transposed (tensor
  engine) for the HBM store.
- After each layer an AllGather over the 8 cores rebuilds the full replica.
- Mean-pool: during the last layer, each [node, feat] output tile is reduced
  into a PSUM[feat, graph] accumulator via a matmul against a graph-selection
  one-hot built from a per-core batch-id input; scale by 1/count, AllReduce,
  then the tiny classifier MLP on every core.
"""
import os
import sys

sys.path.insert(0, "/opt/trn_rl_repo")

import numpy as np

N_NODES = 100000
N_EDGES = 1600000
F = 128
NUM_LAYERS = int(os.environ.get("GNN_LAYERS", "10"))
NUM_GRAPHS = 64
NUM_CLASSES = 2
N_CORES = 8
NPC = N_NODES // N_CORES          # 12500 nodes per core
NT = (NPC + 127) // 128           # 98 dst tiles per core
LAST_W = NPC - (NT - 1) * 128     # 84 nodes in the last tile

_CACHE = {}


def _preprocess(edge_index, batch):
    """Host-side: per-core gather indices / dst offsets in the SBUF layout."""
    src = np.ascontiguousarray(edge_index[0]).astype(np.int64)
    dst = np.ascontiguousarray(edge_index[1]).astype(np.int64)

    order = np.argsort(dst, kind="stable")
    s_src = src[order].astype(np.int32)
    s_dst = dst[order]

    # node-id boundaries of every (core, tile)
    tile_starts = (np.arange(N_CORES)[:, None] * NPC
                   + np.minimum(np.arange(NT + 1)[None, :] * 128, NPC))
    bounds = np.searchsorted(s_dst, tile_starts.ravel()).reshape(N_CORES, NT + 1)
    counts = np.diff(bounds, axis=1)                      # [N_CORES, NT]

    padded = np.maximum(counts.max(axis=0), 1)
    padded = ((padded + 127) // 128) * 128                # per-tile padded len
    nch = (padded // 128).astype(np.int64)                # 128-chunks per tile
    colbase = np.concatenate([[0], np.cumsum(nch)])
    C_total = int(colbase[-1])

    gidx = np.zeros((N_CORES, 128, C_total), dtype=np.int32)
    gdst = np.full((N_CORES, 128, C_total), -1.0, dtype=np.float32)
    for c in range(N_CORES):
        lo, hi = bounds[c, 0], bounds[c, NT]
        e_src = s_src[lo:hi]
        local = s_dst[lo:hi] - c * NPC
        t_e = local // 128
        grp_start = np.repeat(bounds[c, :-1] - lo, counts[c])
        r = np.arange(hi - lo) - grp_start                # rank within tile
        p = r % 128
        col = colbase[t_e] + r // 128
        gidx[c, p, col] = e_src
        gdst[c, p, col] = (local % 128).astype(np.float32)

    # per-core local batch ids, [128, NT], padding rows = -1
    blocal = np.full((N_CORES, 128, NT), -1.0, dtype=np.float32)
    b = np.asarray(batch).astype(np.int64)
    for c in range(N_CORES):
        ids = b[c * NPC:(c + 1) * NPC].astype(np.float32)
        ids = np.concatenate([ids, np.full(NT * 128 - NPC, -1.0, np.float32)])
        blocal[c] = ids.reshape(NT, 128).T

    cnt = np.bincount(b, minlength=NUM_GRAPHS).astype(np.float64)
    inv = (1.0 / np.maximum(cnt, 1.0)).astype(np.float32)
    return gidx, gdst, nch, colbase, blocal, inv


def _build(nch, colbase):
    from concourse import bacc, bass, mybir
    import concourse.tile as tile

    f32 = mybir.dt.float32
    C_total = int(colbase[-1])

    nc = bacc.Bacc("TRN2", target_bir_lowering=False, debug=False,
                   num_devices=N_CORES)

    # ---- I/O ----
    x_in = nc.dram_tensor("x", [N_NODES, F], f32, kind="ExternalInput")
    xT_in = nc.dram_tensor("xT_own", [F, NPC], f32, kind="ExternalInput")
    gidx_in = nc.dram_tensor("gidx", [128, C_total], mybir.dt.int32,
                             kind="ExternalInput")
    gdst_in = nc.dram_tensor("gdst", [128, C_total], f32, kind="ExternalInput")
    bl_in = nc.dram_tensor("blocal", [128, NT], f32, kind="ExternalInput")
    iota_in = nc.dram_tensor("iota", [128, 128], f32, kind="ExternalInput")
    ident_in = nc.dram_tensor("ident", [128, 128], f32, kind="ExternalInput")
    w1_in = nc.dram_tensor("w1cat", [F, NUM_LAYERS * F], f32, kind="ExternalInput")
    w2_in = nc.dram_tensor("w2cat", [F, NUM_LAYERS * F], f32, kind="ExternalInput")
    b1_in = nc.dram_tensor("b1t", [F, NUM_LAYERS], f32, kind="ExternalInput")
    b2_in = nc.dram_tensor("b2t", [F, NUM_LAYERS], f32, kind="ExternalInput")
    eps_in = nc.dram_tensor("epsb", [F, NUM_LAYERS], f32, kind="ExternalInput")
    wc1_in = nc.dram_tensor("wc1", [F, F], f32, kind="ExternalInput")
    bc1_in = nc.dram_tensor("bc1c", [F, 1], f32, kind="ExternalInput")
    wc2_in = nc.dram_tensor("wc2", [F, NUM_CLASSES], f32, kind="ExternalInput")
    bc2_in = nc.dram_tensor("bc2c", [NUM_CLASSES, 1], f32, kind="ExternalInput")
    inv_in = nc.dram_tensor("invc", [128, NUM_GRAPHS], f32, kind="ExternalInput")
    out_t = nc.dram_tensor("logits_t", [NUM_CLASSES, NUM_GRAPHS], f32,
                           kind="ExternalOutput")

    # ---- internal DRAM ----
    x_rep = nc.dram_tensor("x_rep", [N_NODES, F], f32, kind="Internal")
    newx = nc.dram_tensor("newx", [NPC, F], f32, kind="Internal")
    pr_in = nc.dram_tensor("pr_in", [128, NUM_GRAPHS], f32, kind="Internal")
    pr_out = nc.dram_tensor("pr_out", [128, NUM_GRAPHS], f32, kind="Internal")

    rg = [list(range(N_CORES))]

    with tile.TileContext(nc) as tc:
        from contextlib import ExitStack
        ctx = ExitStack()
        const = ctx.enter_context(tc.tile_pool(name="const", bufs=1))
        gpool = ctx.enter_context(tc.tile_pool(name="gather", bufs=3))
        opool = ctx.enter_context(tc.tile_pool(name="onehot", bufs=3))
        wpool = ctx.enter_context(tc.tile_pool(name="work", bufs=3))
        psum = ctx.enter_context(tc.tile_pool(name="psum", bufs=2, space="PSUM"))

        xT_res = const.tile([F, NPC], f32)
        gidx_t = const.tile([128, C_total], mybir.dt.int32)
        gdst_t = const.tile([128, C_total], f32)
        bl_t = const.tile([128, NT], f32)
        iota_t = const.tile([128, 128], f32)
        ident_t = const.tile([128, 128], f32)
        w1_t = const.tile([F, NUM_LAYERS * F], f32)
        w2_t = const.tile([F, NUM_LAYERS * F], f32)
        b1_t = const.tile([F, NUM_LAYERS], f32)
        b2_t = const.tile([F, NUM_LAYERS], f32)
        eps_t = const.tile([F, NUM_LAYERS], f32)
        wc1_t = const.tile([F, F], f32)
        bc1_t = const.tile([F, 1], f32)
        wc2_t = const.tile([F, NUM_CLASSES], f32)
        bc2_t = const.tile([NUM_CLASSES, 1], f32)
        inv_t = const.tile([128, NUM_GRAPHS], f32)

        for tle, src_t in [(xT_res, xT_in), (gdst_t, gdst_in), (bl_t, bl_in),
                           (iota_t, iota_in), (ident_t, ident_in),
                           (w1_t, w1_in), (w2_t, w2_in), (b1_t, b1_in),
                           (b2_t, b2_in), (eps_t, eps_in), (wc1_t, wc1_in),
                           (bc1_t, bc1_in), (wc2_t, wc2_in), (bc2_t, bc2_in),
                           (inv_t, inv_in)]:
            nc.sync.dma_start(tle[:], src_t[:])
        nc.sync.dma_start(gidx_t[:], gidx_in[:])

        pool_ps = psum.tile([F, NUM_GRAPHS], f32, tag="pool", bufs=1)

        for layer in range(NUM_LAYERS):
            src_dram = x_in if layer == 0 else x_rep
            last = layer == NUM_LAYERS - 1
            for t in range(NT):
                tw = 128 if t < NT - 1 else LAST_W
                n = int(nch[t])
                cb = int(colbase[t])
                ts = t * 128

                gbuf = gpool.tile([128, n, F], f32, tag="gbuf")
                for j in range(n):
                    # HW contract: one offset per partition, 128 rows/call
                    nc.gpsimd.indirect_dma_start(
                        out=gbuf[:, j, :],
                        out_offset=None,
                        in_=src_dram[:],
                        in_offset=bass.IndirectOffsetOnAxis(
                            ap=gidx_t[:, cb + j:cb + j + 1], axis=0),
                    )

                oh = opool.tile([128, n, 128], f32, tag="oh")
                nc.vector.tensor_tensor(
                    out=oh[:],
                    in0=gdst_t[:, cb:cb + n, None].to_broadcast([128, n, 128]),
                    in1=iota_t[:, None, :].to_broadcast([128, n, 128]),
                    op=mybir.AluOpType.is_equal,
                )

                aggr = psum.tile([F, 128], f32, tag="aggr", bufs=2)
                for j in range(n):
                    nc.tensor.matmul(aggr[:], gbuf[:, j, :], oh[:, j, :],
                                     start=(j == 0), stop=(j == n - 1))

                xT_sl = xT_res[:, ts:ts + tw]
                h = wpool.tile([F, 128], f32, tag="h")
                nc.vector.tensor_scalar(
                    out=h[:, :tw], in0=xT_sl, scalar1=eps_t[:, layer:layer + 1],
                    scalar2=None, op0=mybir.AluOpType.mult)
                nc.vector.tensor_tensor(
                    out=h[:, :tw], in0=h[:, :tw], in1=aggr[:, :tw],
                    op=mybir.AluOpType.add)

                p1 = psum.tile([F, 128], f32, tag="p1", bufs=1)
                nc.tensor.matmul(p1[:, :tw], w1_t[:, layer * F:(layer + 1) * F],
                                 h[:, :tw], start=True, stop=True)
                r1 = wpool.tile([F, 128], f32, tag="r1")
                nc.scalar.activation(r1[:, :tw], p1[:, :tw],
                                     mybir.ActivationFunctionType.Relu,
                                     bias=b1_t[:, layer:layer + 1])

                p2 = psum.tile([F, 128], f32, tag="p2", bufs=1)
                nc.tensor.matmul(p2[:, :tw], w2_t[:, layer * F:(layer + 1) * F],
                                 r1[:, :tw], start=True, stop=True)

                o = wpool.tile([F, 128], f32, tag="o")
                if layer > 0:
                    nc.vector.tensor_tensor(out=o[:, :tw], in0=p2[:, :tw],
                                            in1=h[:, :tw],
                                            op=mybir.AluOpType.add)
                    nc.scalar.activation(o[:, :tw], o[:, :tw],
                                         mybir.ActivationFunctionType.Relu,
                                         bias=b2_t[:, layer:layer + 1])
                else:
                    nc.scalar.activation(o[:, :tw], p2[:, :tw],
                                         mybir.ActivationFunctionType.Relu,
                                         bias=b2_t[:, layer:layer + 1])
                nc.vector.tensor_tensor(out=xT_sl, in0=o[:, :tw], in1=xT_sl,
                                        op=mybir.AluOpType.add)

                pt = psum.tile([128, F], f32, tag="pt", bufs=2)
                nc.tensor.transpose(out=pt[:tw, :], in_=xT_res[:, ts:ts + tw],
                                    identity=ident_t[:])
                st = wpool.tile([128, F], f32, tag="st")
                nc.vector.tensor_copy(st[:tw, :], pt[:tw, :])
                if not last:
                    nc.sync.dma_start(newx[ts:ts + tw, :], st[:tw, :])
                else:
                    # fold this tile into the pooling accumulator
                    sel = wpool.tile([128, NUM_GRAPHS], f32, tag="sel")
                    nc.vector.tensor_tensor(
                        out=sel[:],
                        in0=bl_t[:, t:t + 1].to_broadcast([128, NUM_GRAPHS]),
                        in1=iota_t[:, :NUM_GRAPHS],
                        op=mybir.AluOpType.is_equal,
                    )
                    nc.tensor.matmul(pool_ps[:], st[:], sel[:],
                                     start=(t == 0), stop=(t == NT - 1))

            if not last:
                nc.gpsimd.collective_compute(
                    "AllGather", mybir.AluOpType.bypass,
                    ins=[newx[:]], outs=[x_rep[:]], replica_groups=rg)

        # ---- mean pool + classifier ----
        pacc = wpool.tile([128, NUM_GRAPHS], f32, tag="pacc")
        nc.vector.tensor_tensor(out=pacc[:], in0=pool_ps[:], in1=inv_t[:],
                                op=mybir.AluOpType.mult)
        nc.sync.dma_start(pr_in[:], pacc[:])
        nc.gpsimd.collective_compute(
            "AllReduce", mybir.AluOpType.add,
            ins=[pr_in[:]], outs=[pr_out[:]], replica_groups=rg)
        pooled = wpool.tile([128, NUM_GRAPHS], f32, tag="pooled")
        nc.sync.dma_start(pooled[:], pr_out[:])

        pc1 = psum.tile([F, NUM_GRAPHS], f32, tag="aggr", bufs=2)
        nc.tensor.matmul(pc1[:], wc1_t[:], pooled[:], start=True, stop=True)
        rc1 = wpool.tile([F, NUM_GRAPHS], f32, tag="rc1")
        nc.scalar.activation(rc1[:], pc1[:], mybir.ActivationFunctionType.Relu,
                             bias=bc1_t[:])
        pc2 = psum.tile([NUM_CLASSES, NUM_GRAPHS], f32, tag="p1", bufs=1)
        nc.tensor.matmul(pc2[:], wc2_t[:], rc1[:], start=True, stop=True)
        lg = wpool.tile([NUM_CLASSES, NUM_GRAPHS], f32, tag="lg")
        nc.vector.tensor_scalar(out=lg[:], in0=pc2[:], scalar1=bc2_t[:],
                                scalar2=None, op0=mybir.AluOpType.add)
        nc.sync.dma_start(out_t[:], lg[:])
        ctx.close()

    nc.compile()
    return nc


def _get_module(nch, colbase):
    key = tuple(nch.tolist())
    if key not in _CACHE:
        _CACHE.clear()
        _CACHE[key] = _build(nch, colbase)
    return _CACHE[key]


def kernel(x, edge_index, batch, eps, W1, b1, W2, b2, Wc1, bc1, Wc2, bc2,
           _trace=False):
    from concourse.bass_utils import run_bass_kernel_spmd

    x = np.ascontiguousarray(np.asarray(x), dtype=np.float32)
    eps = np.asarray(eps, dtype=np.float32)
    W1 = np.asarray(W1, dtype=np.float32)
    b1 = np.asarray(b1, dtype=np.float32)
    W2 = np.asarray(W2, dtype=np.float32)
    b2 = np.asarray(b2, dtype=np.float32)

    gidx, gdst, nch, colbase, blocal, inv = _preprocess(
        np.asarray(edge_index), np.asarray(batch))
    nc = _get_module(nch, colbase)

    L = NUM_LAYERS
    common = {
        "x": x,
        "iota": np.ascontiguousarray(
            np.broadcast_to(np.arange(128, dtype=np.float32), (128, 128))),
        "ident": np.eye(128, dtype=np.float32),
        "w1cat": np.ascontiguousarray(np.concatenate(list(W1[:L]), axis=1)),
        "w2cat": np.ascontiguousarray(np.concatenate(list(W2[:L]), axis=1)),
        "b1t": np.ascontiguousarray(b1[:L].T),
        "b2t": np.ascontiguousarray(b2[:L].T),
        "epsb": np.ascontiguousarray(
            np.broadcast_to(1.0 + eps[:L], (F, L))),
        "wc1": np.ascontiguousarray(np.asarray(Wc1, np.float32)),
        "bc1c": np.ascontiguousarray(np.asarray(bc1, np.float32)[:, None]),
        "wc2": np.ascontiguousarray(np.asarray(Wc2, np.float32)),
        "bc2c": np.ascontiguousarray(np.asarray(bc2, np.float32)[:, None]),
        "invc": np.ascontiguousarray(np.broadcast_to(inv, (128, NUM_GRAPHS))),
    }
    in_maps = []
    for c in range(N_CORES):
        m = dict(common)
        m["xT_own"] = np.ascontiguousarray(x[c * NPC:(c + 1) * NPC].T)
        m["gidx"] = gidx[c]
        m["gdst"] = gdst[c]
        m["blocal"] = blocal[c]
        in_maps.append(m)

    res = run_bass_kernel_spmd(nc, in_maps, core_ids=list(range(N_CORES)),
                               trace=_trace)
    out = np.ascontiguousarray(res.results[0]["logits_t"].T)
    if _trace:
        kernel._last_result = res
    return out



# revision 4
# speedup vs baseline: 12.4769x; 12.4769x over previous
"""GIN message-passing GNN on 8 Trainium2 NeuronCores (Bass/Tile).

v2 strategy (self-contained; shapes hardcoded for the 100k-node / 1.6M-edge /
128-dim / 10-layer / 64-graph problem):

- Nodes are partitioned into 8 contiguous ranges of 12500; each core owns the
  edges whose *destination* falls in its range.
- Node features are replicated in every core's HBM in bf16, split into 4
  "quarter" tensors (<=25600 rows each so gather indices fit int16).
- Per layer, each core gathers x[src] for its ~200k edges with batched
  `dma_gather` calls (one per (group-of-7-dst-tiles, quarter) — 56 calls per
  layer instead of ~1660 per-chunk indirect DMAs, amortizing the ~1us SWDGE
  descriptor-generation overhead that dominated the baseline).
- The segment-sum (scatter-add) is a bf16 matmul per 128-edge chunk:
  PSUM[feat, dst] += gathered[edge, feat]^T @ onehot[edge, dst], with the
  one-hot built on the vector engine (bf16) from a resident dst-offset row.
  Pad slots carry dst=-1 -> all-zero one-hot column.
- The GIN MLP runs fp32 in the transposed [feat, node] orientation; the
  per-core x^T slab stays resident in SBUF (fp32) for exact residual adds.
- Each layer's output tile is transposed (tensor engine), converted to bf16,
  and stored to HBM; four AllGathers (one per quarter, fired as soon as that
  quarter's tiles are done) rebuild the replicas, ping-ponging between two
  tensor sets across layers so collectives overlap next-layer compute.
- Mean-pool: during the last layer each output tile is folded into a
  PSUM[feat, graph] accumulator via a matmul against a graph one-hot;
  scale by 1/count, AllReduce, then the tiny classifier MLP on every core.
"""
import os
import sys

sys.path.insert(0, "/opt/trn_rl_repo")

import numpy as np
import ml_dtypes

BF16 = ml_dtypes.bfloat16

N_NODES = 100000
N_EDGES = 1600000
F = 128
NUM_LAYERS = int(os.environ.get("GNN_LAYERS", "10"))
NUM_GRAPHS = 64
NUM_CLASSES = 2
N_CORES = 8
NPC = N_NODES // N_CORES          # 12500 nodes per core
NT = (NPC + 127) // 128           # 98 dst tiles per core
LAST_W = NPC - (NT - 1) * 128     # 84 nodes in the last tile

NQ = 4                            # quarters (int16 index range)
QT = [0, 25, 49, 73, 98]          # tile boundaries of quarters
QROW = [0, 3200, 6272, 9344, 12500]
QSZ = [3200, 3072, 3072, 3156]    # rows per core per quarter
GRP = 7                           # dst tiles per gather group
NGRP = NT // GRP                  # 14 groups (98 = 14*7)

_CACHE = {}


def _preprocess(edge_index, batch):
    """Host-side: per-core int16 gather indices (quartered layout), per-slot
    dst offsets, chunk layout, batch ids, inverse counts."""
    src = np.ascontiguousarray(edge_index[0]).astype(np.int64)
    dst = np.ascontiguousarray(edge_index[1]).astype(np.int64)

    order = np.argsort(dst, kind="stable")
    s_src = src[order]
    s_dst = dst[order]

    # quartered row id of each source node
    sc = s_src // NPC
    sr = s_src % NPC
    sq = np.searchsorted(QROW, sr, side="right") - 1        # quarter 0..3
    qsz = np.asarray(QSZ, np.int64)
    qrow = np.asarray(QROW[:4], np.int64)
    s_lrow = (sc * qsz[sq] + (sr - qrow[sq])).astype(np.int64)

    # core / tile / local-dst of each edge
    dcore = s_dst // NPC
    dloc = s_dst % NPC
    dtile = dloc // 128
    doff = dloc % 128

    # counts per (core, tile, quarter)
    key_global = (dcore * NT + dtile) * NQ + sq
    cnt = np.bincount(key_global, minlength=N_CORES * NT * NQ)
    cnt = cnt.reshape(N_CORES, NT, NQ)
    nchq = (np.maximum(cnt.max(axis=0), 0) + 127) // 128      # [NT, NQ] chunks

    # column layout: group-major, quarter-major inside group, tile inside
    colbase = np.zeros((NT, NQ), dtype=np.int64)
    call_start = np.zeros((NGRP, NQ), dtype=np.int64)
    call_cols = np.zeros((NGRP, NQ), dtype=np.int64)
    cb = 0
    for g in range(NGRP):
        for q in range(NQ):
            call_start[g, q] = cb
            for t in range(g * GRP, (g + 1) * GRP):
                colbase[t, q] = cb
                cb += nchq[t, q]
            call_cols[g, q] = cb - call_start[g, q]
    C_total = int(cb)

    gidx = np.zeros((N_CORES, 128, C_total * 8), dtype=np.int16)
    gdst = np.full((N_CORES, 128, C_total), -1.0, dtype=np.float32)

    colbase_flat = colbase.reshape(-1)          # [NT*NQ]
    # call start of the group owning each (t, q)
    cs_of_tq = np.zeros((NT, NQ), dtype=np.int64)
    for t in range(NT):
        for q in range(NQ):
            cs_of_tq[t, q] = call_start[t // GRP, q]
    cs_flat = cs_of_tq.reshape(-1)

    for c in range(N_CORES):
        m = dcore == c
        key = (dtile[m] * NQ + sq[m]).astype(np.int64)       # [nE_c]
        lrow = s_lrow[m]
        dof = doff[m]
        o2 = np.argsort(key, kind="stable")
        key = key[o2]
        lrow = lrow[o2]
        dof = dof[o2]
        ccnt = np.bincount(key, minlength=NT * NQ)
        starts = np.concatenate([[0], np.cumsum(ccnt)])
        rank = np.arange(key.size) - starts[key]
        col = colbase_flat[key] + rank // 128
        part = rank % 128
        gdst[c, part, col] = dof.astype(np.float32)
        # index slot within the gather call
        s_slot = (colbase_flat[key] - cs_flat[key]) * 128 + rank
        ip = s_slot % 16
        ic = cs_flat[key] * 8 + s_slot // 16
        gidx[c, ip, ic] = lrow.astype(np.int16)
        # the 16-partition wrap must be replicated to all 8 Q7-core stripes
        gidx[c] = np.tile(gidx[c, :16], (8, 1))

    # per-core local batch ids, [128, NT], padding rows = -1
    blocal = np.full((N_CORES, 128, NT), -1.0, dtype=np.float32)
    b = np.asarray(batch).astype(np.int64)
    for c in range(N_CORES):
        ids = b[c * NPC:(c + 1) * NPC].astype(np.float32)
        ids = np.concatenate([ids, np.full(NT * 128 - NPC, -1.0, np.float32)])
        blocal[c] = ids.reshape(NT, 128).T

    cntg = np.bincount(b, minlength=NUM_GRAPHS).astype(np.float64)
    inv = (1.0 / np.maximum(cntg, 1.0)).astype(np.float32)
    return gidx, gdst, nchq, colbase, call_start, call_cols, blocal, inv


def _build(nchq, colbase, call_start, call_cols):
    from concourse import bacc, bass, mybir
    import concourse.tile as tile

    f32 = mybir.dt.float32
    b16 = mybir.dt.bfloat16
    C_total = int(call_start[-1, -1] + call_cols[-1, -1])
    # gather-buffer columns per group (group g covers its 4 calls' columns)
    grp_cols = call_cols.sum(axis=1)            # [NGRP]
    CG = int(grp_cols.max())
    NCH_T = nchq.sum(axis=1)                    # chunks per tile
    NCH_MAX = int(NCH_T.max())

    nc = bacc.Bacc("TRN2", target_bir_lowering=False, debug=False,
                   num_devices=N_CORES)

    # ---- I/O ----
    x0q = [nc.dram_tensor(f"x0q{q}", [N_CORES * QSZ[q], F], b16,
                          kind="ExternalInput") for q in range(NQ)]
    xT_in = nc.dram_tensor("xT_own", [F, NPC], f32, kind="ExternalInput")
    gidx_in = nc.dram_tensor("gidx", [128, C_total * 8], mybir.dt.int16,
                             kind="ExternalInput")
    gdst_in = nc.dram_tensor("gdst", [128, C_total], b16, kind="ExternalInput")
    bl_in = nc.dram_tensor("blocal", [128, NT], b16, kind="ExternalInput")
    iota_in = nc.dram_tensor("iota", [128, 128], b16, kind="ExternalInput")
    ident_in = nc.dram_tensor("ident", [128, 128], f32, kind="ExternalInput")
    w1_in = nc.dram_tensor("w1cat", [F, NUM_LAYERS * F], f32, kind="ExternalInput")
    w2_in = nc.dram_tensor("w2cat", [F, NUM_LAYERS * F], f32, kind="ExternalInput")
    b1_in = nc.dram_tensor("b1t", [F, NUM_LAYERS], f32, kind="ExternalInput")
    b2_in = nc.dram_tensor("b2t", [F, NUM_LAYERS], f32, kind="ExternalInput")
    eps_in = nc.dram_tensor("epsb", [F, NUM_LAYERS], f32, kind="ExternalInput")
    wc1_in = nc.dram_tensor("wc1", [F, F], f32, kind="ExternalInput")
    bc1_in = nc.dram_tensor("bc1c", [F, 1], f32, kind="ExternalInput")
    wc2_in = nc.dram_tensor("wc2", [F, NUM_CLASSES], f32, kind="ExternalInput")
    bc2_in = nc.dram_tensor("bc2c", [NUM_CLASSES, 1], f32, kind="ExternalInput")
    inv_in = nc.dram_tensor("invc", [128, NUM_GRAPHS], f32, kind="ExternalInput")
    out_t = nc.dram_tensor("logits_t", [NUM_CLASSES, NUM_GRAPHS], f32,
                           kind="ExternalOutput")

    # ---- internal DRAM ----
    newx = nc.dram_tensor("newx", [NPC, F], b16, kind="Internal")
    xA = [nc.dram_tensor(f"xAq{q}", [N_CORES * QSZ[q], F], b16,
                         kind="Internal", addr_space="Shared")
          for q in range(NQ)]
    xB = [nc.dram_tensor(f"xBq{q}", [N_CORES * QSZ[q], F], b16,
                         kind="Internal", addr_space="Shared")
          for q in range(NQ)]
    pr_in = nc.dram_tensor("pr_in", [128, NUM_GRAPHS], f32, kind="Internal")
    pr_out = nc.dram_tensor("pr_out", [128, NUM_GRAPHS], f32, kind="Internal",
                            addr_space="Shared")

    rg = [list(range(N_CORES))]

    with tile.TileContext(nc) as tc:
        from contextlib import ExitStack
        ctx = ExitStack()
        const = ctx.enter_context(tc.tile_pool(name="const", bufs=1))
        gpool = ctx.enter_context(tc.tile_pool(name="gather", bufs=2))
        opool = ctx.enter_context(tc.tile_pool(name="onehot", bufs=3))
        wpool = ctx.enter_context(tc.tile_pool(name="work", bufs=3))
        psum = ctx.enter_context(tc.tile_pool(name="psum", bufs=2, space="PSUM"))

        xT_res = const.tile([F, NPC], f32)
        gidx_t = const.tile([128, C_total * 8], mybir.dt.int16)
        gdst_t = const.tile([128, C_total], b16)
        bl_t = const.tile([128, NT], b16)
        iota_t = const.tile([128, 128], b16)
        ident_t = const.tile([128, 128], f32)
        w1_t = const.tile([F, NUM_LAYERS * F], f32)
        w2_t = const.tile([F, NUM_LAYERS * F], f32)
        b1_t = const.tile([F, NUM_LAYERS], f32)
        b2_t = const.tile([F, NUM_LAYERS], f32)
        eps_t = const.tile([F, NUM_LAYERS], f32)
        wc1_t = const.tile([F, F], f32)
        bc1_t = const.tile([F, 1], f32)
        wc2_t = const.tile([F, NUM_CLASSES], f32)
        bc2_t = const.tile([NUM_CLASSES, 1], f32)
        inv_t = const.tile([128, NUM_GRAPHS], f32)

        for tle, src_t in [(xT_res, xT_in), (gdst_t, gdst_in), (bl_t, bl_in),
                           (iota_t, iota_in), (ident_t, ident_in),
                           (w1_t, w1_in), (w2_t, w2_in), (b1_t, b1_in),
                           (b2_t, b2_in), (eps_t, eps_in), (wc1_t, wc1_in),
                           (bc1_t, bc1_in), (wc2_t, wc2_in), (bc2_t, bc2_in),
                           (inv_t, inv_in)]:
            nc.sync.dma_start(tle[:], src_t[:])
        nc.sync.dma_start(gidx_t[:], gidx_in[:])

        pool_ps = psum.tile([F, NUM_GRAPHS], f32, tag="pool", bufs=1)

        for layer in range(NUM_LAYERS):
            if layer == 0:
                srcs = x0q
            else:
                srcs = xA if (layer - 1) % 2 == 0 else xB
            wset = xA if layer % 2 == 0 else xB
            last = layer == NUM_LAYERS - 1

            for g in range(NGRP):
                gcs = int(call_start[g, 0])      # first column of this group
                gbuf = gpool.tile([128, CG, 128], b16, tag="gbuf")
                for q in range(NQ):
                    cols = int(call_cols[g, q])
                    # Q7 dma_gather handles at most 1024 indices (8 columns)
                    # per call — split each (group, quarter) region.
                    for sc in range(0, cols, 8):
                        w = min(8, cols - sc)
                        cs = int(call_start[g, q]) + sc
                        lc = cs - gcs
                        nidx = 128 * w
                        nc.gpsimd.dma_gather(
                            gbuf[:, lc:lc + w, :],
                            srcs[q][:],
                            gidx_t[:, cs * 8:(cs + w) * 8],
                            nidx, nidx, F)

                for t in range(g * GRP, (g + 1) * GRP):
                    tw = 128 if t < NT - 1 else LAST_W
                    ts = t * 128
                    ncht = int(NCH_T[t])

                    # one-hot for all of tile t's chunks (4 quarter ranges)
                    oh = opool.tile([128, NCH_MAX, 128], b16, tag="oh")
                    off = 0
                    for q in range(NQ):
                        n = int(nchq[t, q])
                        if n == 0:
                            continue
                        cb = int(colbase[t, q])
                        nc.vector.tensor_tensor(
                            out=oh[:, off:off + n, :],
                            in0=gdst_t[:, cb:cb + n, None].to_broadcast(
                                [128, n, 128]),
                            in1=iota_t[:, None, :].to_broadcast([128, n, 128]),
                            op=mybir.AluOpType.is_equal,
                        )
                        off += n

                    aggr = psum.tile([F, 128], f32, tag="aggr", bufs=2)
                    j = 0
                    for q in range(NQ):
                        n = int(nchq[t, q])
                        lc = int(colbase[t, q]) - gcs
                        for k in range(n):
                            nc.tensor.matmul(aggr[:], gbuf[:, lc + k, :],
                                             oh[:, j, :],
                                             start=(j == 0),
                                             stop=(j == ncht - 1))
                            j += 1

                    xT_sl = xT_res[:, ts:ts + tw]
                    h = wpool.tile([F, 128], f32, tag="h")
                    nc.vector.tensor_scalar(
                        out=h[:, :tw], in0=xT_sl,
                        scalar1=eps_t[:, layer:layer + 1],
                        scalar2=None, op0=mybir.AluOpType.mult)
                    nc.vector.tensor_tensor(
                        out=h[:, :tw], in0=h[:, :tw], in1=aggr[:, :tw],
                        op=mybir.AluOpType.add)

                    p1 = psum.tile([F, 128], f32, tag="p1", bufs=1)
                    nc.tensor.matmul(p1[:, :tw],
                                     w1_t[:, layer * F:(layer + 1) * F],
                                     h[:, :tw], start=True, stop=True)
                    r1 = wpool.tile([F, 128], f32, tag="r1")
                    nc.scalar.activation(r1[:, :tw], p1[:, :tw],
                                         mybir.ActivationFunctionType.Relu,
                                         bias=b1_t[:, layer:layer + 1])

                    p2 = psum.tile([F, 128], f32, tag="p2", bufs=1)
                    nc.tensor.matmul(p2[:, :tw],
                                     w2_t[:, layer * F:(layer + 1) * F],
                                     r1[:, :tw], start=True, stop=True)

                    o = wpool.tile([F, 128], f32, tag="o")
                    if layer > 0:
                        nc.vector.tensor_tensor(out=o[:, :tw], in0=p2[:, :tw],
                                                in1=h[:, :tw],
                                                op=mybir.AluOpType.add)
                        nc.scalar.activation(o[:, :tw], o[:, :tw],
                                             mybir.ActivationFunctionType.Relu,
                                             bias=b2_t[:, layer:layer + 1])
                    else:
                        nc.scalar.activation(o[:, :tw], p2[:, :tw],
                                             mybir.ActivationFunctionType.Relu,
                                             bias=b2_t[:, layer:layer + 1])
                    nc.vector.tensor_tensor(out=xT_sl, in0=o[:, :tw],
                                            in1=xT_sl,
                                            op=mybir.AluOpType.add)

                    pt = psum.tile([128, F], f32, tag="pt", bufs=2)
                    nc.tensor.transpose(out=pt[:tw, :],
                                        in_=xT_res[:, ts:ts + tw],
                                        identity=ident_t[:])
                    st = wpool.tile([128, F], b16, tag="st")
                    nc.vector.tensor_copy(st[:tw, :], pt[:tw, :])
                    if not last:
                        nc.sync.dma_start(newx[ts:ts + tw, :], st[:tw, :])
                    else:
                        sel = wpool.tile([128, NUM_GRAPHS], b16, tag="sel")
                        nc.vector.tensor_tensor(
                            out=sel[:],
                            in0=bl_t[:, t:t + 1].to_broadcast(
                                [128, NUM_GRAPHS]),
                            in1=iota_t[:, :NUM_GRAPHS],
                            op=mybir.AluOpType.is_equal,
                        )
                        nc.tensor.matmul(pool_ps[:], st[:], sel[:],
                                         start=(t == 0), stop=(t == NT - 1))

                    # fire this quarter's AllGather as soon as its tiles done
                    if not last:
                        for q in range(NQ):
                            if t == QT[q + 1] - 1:
                                nc.gpsimd.collective_compute(
                                    "AllGather", mybir.AluOpType.bypass,
                                    ins=[newx[QROW[q]:QROW[q + 1], :]],
                                    outs=[wset[q][:]], replica_groups=rg)

        # ---- mean pool + classifier ----
        pacc = wpool.tile([128, NUM_GRAPHS], f32, tag="pacc")
        nc.vector.tensor_tensor(out=pacc[:], in0=pool_ps[:], in1=inv_t[:],
                                op=mybir.AluOpType.mult)
        nc.sync.dma_start(pr_in[:], pacc[:])
        nc.gpsimd.collective_compute(
            "AllReduce", mybir.AluOpType.add,
            ins=[pr_in[:]], outs=[pr_out[:]], replica_groups=rg)
        pooled = wpool.tile([128, NUM_GRAPHS], f32, tag="pooled")
        nc.sync.dma_start(pooled[:], pr_out[:])

        pc1 = psum.tile([F, NUM_GRAPHS], f32, tag="aggr", bufs=2)
        nc.tensor.matmul(pc1[:], wc1_t[:], pooled[:], start=True, stop=True)
        rc1 = wpool.tile([F, NUM_GRAPHS], f32, tag="rc1")
        nc.scalar.activation(rc1[:], pc1[:], mybir.ActivationFunctionType.Relu,
                             bias=bc1_t[:])
        pc2 = psum.tile([NUM_CLASSES, NUM_GRAPHS], f32, tag="p1", bufs=1)
        nc.tensor.matmul(pc2[:], wc2_t[:], rc1[:], start=True, stop=True)
        lg = wpool.tile([NUM_CLASSES, NUM_GRAPHS], f32, tag="lg")
        nc.vector.tensor_scalar(out=lg[:], in0=pc2[:], scalar1=bc2_t[:],
                                scalar2=None, op0=mybir.AluOpType.add)
        nc.sync.dma_start(out_t[:], lg[:])
        ctx.close()

    nc.compile()
    return nc


def _get_module(nchq, colbase, call_start, call_cols):
    key = tuple(nchq.reshape(-1).tolist())
    if key not in _CACHE:
        _CACHE.clear()
        _CACHE[key] = _build(nchq, colbase, call_start, call_cols)
    return _CACHE[key]


def kernel(x, edge_index, batch, eps, W1, b1, W2, b2, Wc1, bc1, Wc2, bc2,
           _trace=False):
    from concourse.bass_utils import run_bass_kernel_spmd

    x = np.ascontiguousarray(np.asarray(x), dtype=np.float32)
    eps = np.asarray(eps, dtype=np.float32)
    W1 = np.asarray(W1, dtype=np.float32)
    b1 = np.asarray(b1, dtype=np.float32)
    W2 = np.asarray(W2, dtype=np.float32)
    b2 = np.asarray(b2, dtype=np.float32)

    (gidx, gdst, nchq, colbase, call_start, call_cols,
     blocal, inv) = _preprocess(np.asarray(edge_index), np.asarray(batch))
    nc = _get_module(nchq, colbase, call_start, call_cols)

    # quartered bf16 replicas of the input features
    xbf = x.astype(BF16)
    x0q = []
    for q in range(NQ):
        parts = [xbf[c * NPC + QROW[q]: c * NPC + QROW[q + 1]]
                 for c in range(N_CORES)]
        x0q.append(np.ascontiguousarray(np.concatenate(parts, axis=0)))

    L = NUM_LAYERS
    common = {
        "iota": np.ascontiguousarray(
            np.broadcast_to(np.arange(128, dtype=np.float32),
                            (128, 128))).astype(BF16),
        "ident": np.eye(128, dtype=np.float32),
        "w1cat": np.ascontiguousarray(np.concatenate(list(W1[:L]), axis=1)),
        "w2cat": np.ascontiguousarray(np.concatenate(list(W2[:L]), axis=1)),
        "b1t": np.ascontiguousarray(b1[:L].T),
        "b2t": np.ascontiguousarray(b2[:L].T),
        "epsb": np.ascontiguousarray(
            np.broadcast_to(1.0 + eps[:L], (F, L))),
        "wc1": np.ascontiguousarray(np.asarray(Wc1, np.float32)),
        "bc1c": np.ascontiguousarray(np.asarray(bc1, np.float32)[:, None]),
        "wc2": np.ascontiguousarray(np.asarray(Wc2, np.float32)),
        "bc2c": np.ascontiguousarray(np.asarray(bc2, np.float32)[:, None]),
        "invc": np.ascontiguousarray(np.broadcast_to(inv, (128, NUM_GRAPHS))),
    }
    for q in range(NQ):
        common[f"x0q{q}"] = x0q[q]

    in_maps = []
    for c in range(N_CORES):
        m = dict(common)
        m["xT_own"] = np.ascontiguousarray(x[c * NPC:(c + 1) * NPC].T)
        m["gidx"] = gidx[c]
        m["gdst"] = gdst[c].astype(BF16)
        m["blocal"] = blocal[c].astype(BF16)
        in_maps.append(m)

    res = run_bass_kernel_spmd(nc, in_maps, core_ids=list(range(N_CORES)),
                               trace=_trace)
    out = np.ascontiguousarray(res.results[0]["logits_t"].T)
    if _trace:
        kernel._last_result = res
    return out


# revision 9
# speedup vs baseline: 25.8293x; 2.0702x over previous
"""GIN message-passing GNN on 8 Trainium2 NeuronCores (Bass/Tile).

v2 strategy (self-contained; shapes hardcoded for the 100k-node / 1.6M-edge /
128-dim / 10-layer / 64-graph problem):

- Nodes are partitioned into 8 contiguous ranges of 12500; each core owns the
  edges whose *destination* falls in its range.
- Node features are replicated in every core's HBM in bf16, split into 4
  "quarter" tensors (<=25600 rows each so gather indices fit int16).
- Per layer, each core gathers x[src] for its ~200k edges with batched
  `dma_gather` calls (one per (group-of-7-dst-tiles, quarter) — 56 calls per
  layer instead of ~1660 per-chunk indirect DMAs, amortizing the ~1us SWDGE
  descriptor-generation overhead that dominated the baseline).
- The segment-sum (scatter-add) is a bf16 matmul per 128-edge chunk:
  PSUM[feat, dst] += gathered[edge, feat]^T @ onehot[edge, dst], with the
  one-hot built on the vector engine (bf16) from a resident dst-offset row.
  Pad slots carry dst=-1 -> all-zero one-hot column.
- The GIN MLP runs fp32 in the transposed [feat, node] orientation; the
  per-core x^T slab stays resident in SBUF (fp32) for exact residual adds.
- Each layer's output tile is transposed (tensor engine), converted to bf16,
  and stored to HBM; four AllGathers (one per quarter, fired as soon as that
  quarter's tiles are done) rebuild the replicas, ping-ponging between two
  tensor sets across layers so collectives overlap next-layer compute.
- Mean-pool: during the last layer each output tile is folded into a
  PSUM[feat, graph] accumulator via a matmul against a graph one-hot;
  scale by 1/count, AllReduce, then the tiny classifier MLP on every core.
"""
import os
import sys

sys.path.insert(0, "/opt/trn_rl_repo")

import numpy as np
import ml_dtypes

BF16 = ml_dtypes.bfloat16

N_NODES = 100000
N_EDGES = 1600000
F = 128
NUM_LAYERS = int(os.environ.get("GNN_LAYERS", "10"))
NUM_GRAPHS = 64
NUM_CLASSES = 2
N_CORES = 8
NPC = N_NODES // N_CORES          # 12500 nodes per core
NT = (NPC + 127) // 128           # 98 dst tiles per core
LAST_W = NPC - (NT - 1) * 128     # 84 nodes in the last tile

NQ = 4                            # quarters (int16 index range)
QT = [0, 25, 49, 73, 98]          # tile boundaries of quarters
QROW = [0, 3200, 6272, 9344, 12500]
QSZ = [3200, 3072, 3072, 3156]    # rows per core per quarter
GRP = 7                           # dst tiles per gather group
NGRP = NT // GRP                  # 14 groups (98 = 14*7)

_CACHE = {}


def _preprocess(edge_index, batch):
    """Host-side: per-core int16 gather indices (quartered layout), per-slot
    dst offsets, chunk layout, batch ids, inverse counts."""
    src = np.ascontiguousarray(edge_index[0]).astype(np.int64)
    dst = np.ascontiguousarray(edge_index[1]).astype(np.int64)

    order = np.argsort(dst, kind="stable")
    s_src = src[order]
    s_dst = dst[order]

    # quartered row id of each source node
    sc = s_src // NPC
    sr = s_src % NPC
    sq = np.searchsorted(QROW, sr, side="right") - 1        # quarter 0..3
    qsz = np.asarray(QSZ, np.int64)
    qrow = np.asarray(QROW[:4], np.int64)
    s_lrow = (sc * qsz[sq] + (sr - qrow[sq])).astype(np.int64)

    # core / tile / local-dst of each edge
    dcore = s_dst // NPC
    dloc = s_dst % NPC
    dtile = dloc // 128
    doff = dloc % 128

    # counts per (core, tile, quarter)
    key_global = (dcore * NT + dtile) * NQ + sq
    cnt = np.bincount(key_global, minlength=N_CORES * NT * NQ)
    cnt = cnt.reshape(N_CORES, NT, NQ)
    nchq = (np.maximum(cnt.max(axis=0), 0) + 127) // 128      # [NT, NQ] chunks

    # column layout: group-major, quarter-major inside group, tile inside
    colbase = np.zeros((NT, NQ), dtype=np.int64)
    call_start = np.zeros((NGRP, NQ), dtype=np.int64)
    call_cols = np.zeros((NGRP, NQ), dtype=np.int64)
    cb = 0
    for g in range(NGRP):
        for q in range(NQ):
            call_start[g, q] = cb
            for t in range(g * GRP, (g + 1) * GRP):
                colbase[t, q] = cb
                cb += nchq[t, q]
            call_cols[g, q] = cb - call_start[g, q]
    C_total = int(cb)

    gidx = np.zeros((N_CORES, 128, C_total * 8), dtype=np.int16)
    gdst = np.full((N_CORES, 128, C_total), -1.0, dtype=np.float32)

    # tile-major column permutation: chunk (t, q, k) lives at gbuf column
    # colbase[t, q] + k but its one-hot column is tmbase[t] + sum_q' nchq + k
    ncht = nchq.sum(axis=1)
    tmbase = np.concatenate([[0], np.cumsum(ncht)])
    tm_of_col = np.zeros(C_total, dtype=np.int64)
    for t in range(NT):
        off = 0
        for q in range(NQ):
            n = int(nchq[t, q])
            tm_of_col[colbase[t, q]:colbase[t, q] + n] = (
                tmbase[t] + off + np.arange(n))
            off += n

    colbase_flat = colbase.reshape(-1)          # [NT*NQ]
    # call start of the group owning each (t, q)
    cs_of_tq = np.zeros((NT, NQ), dtype=np.int64)
    for t in range(NT):
        for q in range(NQ):
            cs_of_tq[t, q] = call_start[t // GRP, q]
    cs_flat = cs_of_tq.reshape(-1)

    for c in range(N_CORES):
        m = dcore == c
        key = (dtile[m] * NQ + sq[m]).astype(np.int64)       # [nE_c]
        lrow = s_lrow[m]
        dof = doff[m]
        o2 = np.argsort(key, kind="stable")
        key = key[o2]
        lrow = lrow[o2]
        dof = dof[o2]
        ccnt = np.bincount(key, minlength=NT * NQ)
        starts = np.concatenate([[0], np.cumsum(ccnt)])
        rank = np.arange(key.size) - starts[key]
        col = colbase_flat[key] + rank // 128
        part = rank % 128
        gdst[c, part, tm_of_col[col]] = dof.astype(np.float32)
        # index slot within the gather call
        s_slot = (colbase_flat[key] - cs_flat[key]) * 128 + rank
        ip = s_slot % 16
        ic = cs_flat[key] * 8 + s_slot // 16
        gidx[c, ip, ic] = lrow.astype(np.int16)
        # the 16-partition wrap must be replicated to all 8 Q7-core stripes
        gidx[c] = np.tile(gidx[c, :16], (8, 1))

    # per-core local batch ids, [128, NT], padding rows = -1
    blocal = np.full((N_CORES, 128, NT), -1.0, dtype=np.float32)
    b = np.asarray(batch).astype(np.int64)
    for c in range(N_CORES):
        ids = b[c * NPC:(c + 1) * NPC].astype(np.float32)
        ids = np.concatenate([ids, np.full(NT * 128 - NPC, -1.0, np.float32)])
        blocal[c] = ids.reshape(NT, 128).T

    cntg = np.bincount(b, minlength=NUM_GRAPHS).astype(np.float64)
    inv = (1.0 / np.maximum(cntg, 1.0)).astype(np.float32)
    return gidx, gdst, nchq, colbase, call_start, call_cols, blocal, inv


def _build(nchq, colbase, call_start, call_cols):
    from concourse import bacc, bass, mybir
    import concourse.tile as tile

    f32 = mybir.dt.float32
    b16 = mybir.dt.bfloat16
    C_total = int(call_start[-1, -1] + call_cols[-1, -1])
    # gather-buffer columns per group (group g covers its 4 calls' columns)
    grp_cols = call_cols.sum(axis=1)            # [NGRP]
    CG = int(grp_cols.max())
    NCH_T = nchq.sum(axis=1)                    # chunks per tile
    NCH_MAX = int(NCH_T.max())

    nc = bacc.Bacc("TRN2", target_bir_lowering=False, debug=False,
                   num_devices=N_CORES, num_swdge_queues=4)

    # ---- I/O ----
    x0q = [nc.dram_tensor(f"x0q{q}", [N_CORES * QSZ[q], F], b16,
                          kind="ExternalInput") for q in range(NQ)]
    xT_in = nc.dram_tensor("xT_own", [F, NPC], f32, kind="ExternalInput")
    gidx_in = nc.dram_tensor("gidx", [128, C_total * 8], mybir.dt.int16,
                             kind="ExternalInput")
    gdst_in = nc.dram_tensor("gdst", [128, C_total], b16, kind="ExternalInput")
    bl_in = nc.dram_tensor("blocal", [128, NT], b16, kind="ExternalInput")
    iota_in = nc.dram_tensor("iota", [128, 128], b16, kind="ExternalInput")
    ident_in = nc.dram_tensor("ident", [128, 128], f32, kind="ExternalInput")
    w1_in = nc.dram_tensor("w1cat", [F, NUM_LAYERS * F], f32, kind="ExternalInput")
    w2_in = nc.dram_tensor("w2cat", [F, NUM_LAYERS * F], f32, kind="ExternalInput")
    b1_in = nc.dram_tensor("b1t", [F, NUM_LAYERS], f32, kind="ExternalInput")
    b2_in = nc.dram_tensor("b2t", [F, NUM_LAYERS], f32, kind="ExternalInput")
    eps_in = nc.dram_tensor("epsb", [F, NUM_LAYERS], f32, kind="ExternalInput")
    wc1_in = nc.dram_tensor("wc1", [F, F], f32, kind="ExternalInput")
    bc1_in = nc.dram_tensor("bc1c", [F, 1], f32, kind="ExternalInput")
    wc2_in = nc.dram_tensor("wc2", [F, NUM_CLASSES], f32, kind="ExternalInput")
    bc2_in = nc.dram_tensor("bc2c", [NUM_CLASSES, 1], f32, kind="ExternalInput")
    inv_in = nc.dram_tensor("invc", [128, NUM_GRAPHS], f32, kind="ExternalInput")
    out_t = nc.dram_tensor("logits_t", [NUM_CLASSES, NUM_GRAPHS], f32,
                           kind="ExternalOutput")

    # ---- internal DRAM ----
    newx = nc.dram_tensor("newx", [NPC, F], b16, kind="Internal")
    xA = [nc.dram_tensor(f"xAq{q}", [N_CORES * QSZ[q], F], b16,
                         kind="Internal", addr_space="Shared")
          for q in range(NQ)]
    xB = [nc.dram_tensor(f"xBq{q}", [N_CORES * QSZ[q], F], b16,
                         kind="Internal", addr_space="Shared")
          for q in range(NQ)]
    pr_in = nc.dram_tensor("pr_in", [128, NUM_GRAPHS], f32, kind="Internal")
    pr_out = nc.dram_tensor("pr_out", [128, NUM_GRAPHS], f32, kind="Internal",
                            addr_space="Shared")

    rg = [list(range(N_CORES))]

    with tile.TileContext(nc) as tc:
        from contextlib import ExitStack
        ctx = ExitStack()
        const = ctx.enter_context(tc.tile_pool(name="const", bufs=1))
        gpool = ctx.enter_context(tc.tile_pool(name="gather", bufs=2))
        opool = ctx.enter_context(tc.tile_pool(name="onehot", bufs=3))
        wpool = ctx.enter_context(tc.tile_pool(name="work", bufs=3))
        psum = ctx.enter_context(tc.tile_pool(name="psum", bufs=2, space="PSUM"))

        xT_res = const.tile([F, NPC], f32)
        gidx_t = const.tile([128, C_total * 8], mybir.dt.int16)
        gdst_t = const.tile([128, C_total], b16)
        bl_t = const.tile([128, NT], b16)
        iota_t = const.tile([128, 128], b16)
        ident_t = const.tile([128, 128], f32)
        w1_t = const.tile([F, NUM_LAYERS * F], f32)
        w2_t = const.tile([F, NUM_LAYERS * F], f32)
        b1_t = const.tile([F, NUM_LAYERS], f32)
        b2_t = const.tile([F, NUM_LAYERS], f32)
        eps_t = const.tile([F, NUM_LAYERS], f32)
        wc1_t = const.tile([F, F], f32)
        bc1_t = const.tile([F, 1], f32)
        wc2_t = const.tile([F, NUM_CLASSES], f32)
        bc2_t = const.tile([NUM_CLASSES, 1], f32)
        inv_t = const.tile([128, NUM_GRAPHS], f32)

        for tle, src_t in [(xT_res, xT_in), (gdst_t, gdst_in), (bl_t, bl_in),
                           (iota_t, iota_in), (ident_t, ident_in),
                           (w1_t, w1_in), (w2_t, w2_in), (b1_t, b1_in),
                           (b2_t, b2_in), (eps_t, eps_in), (wc1_t, wc1_in),
                           (bc1_t, bc1_in), (wc2_t, wc2_in), (bc2_t, bc2_in),
                           (inv_t, inv_in)]:
            nc.sync.dma_start(tle[:], src_t[:])
        nc.sync.dma_start(gidx_t[:], gidx_in[:])

        pool_ps = psum.tile([F, NUM_GRAPHS], f32, tag="pool", bufs=1)

        gcall = 0
        for layer in range(NUM_LAYERS):
            if layer == 0:
                srcs = x0q
            else:
                srcs = xA if (layer - 1) % 2 == 0 else xB
            wset = xA if layer % 2 == 0 else xB
            last = layer == NUM_LAYERS - 1

            for g in range(NGRP):
                gcs = int(call_start[g, 0])      # first column of this group
                gbuf = gpool.tile([128, CG, 128], b16, tag="gbuf")
                for q in range(NQ):
                    cols = int(call_cols[g, q])
                    # Q7 dma_gather handles at most 1024 indices (8 columns)
                    # per call — split each (group, quarter) region, and
                    # round-robin the calls over the 4 SWDGE queues so their
                    # transfers drain through different SDMA rings.
                    for sc in range(0, cols, 8):
                        w = min(8, cols - sc)
                        cs = int(call_start[g, q]) + sc
                        lc = cs - gcs
                        nidx = 128 * w
                        nc.gpsimd.dma_gather(
                            gbuf[:, lc:lc + w, :],
                            srcs[q][:],
                            gidx_t[:, cs * 8:(cs + w) * 8],
                            nidx, nidx, F,
                            queue_num=(gcall % 4))
                        gcall += 1

                for t in range(g * GRP, (g + 1) * GRP):
                    tw = 128 if t < NT - 1 else LAST_W
                    ts = t * 128
                    ncht = int(NCH_T[t])

                    # one-hot for all of tile t's chunks (4 quarter ranges)
                    oh = opool.tile([128, NCH_MAX, 128], b16, tag="oh")
                    off = 0
                    for q in range(NQ):
                        n = int(nchq[t, q])
                        if n == 0:
                            continue
                        cb = int(colbase[t, q])
                        nc.vector.tensor_tensor(
                            out=oh[:, off:off + n, :],
                            in0=gdst_t[:, cb:cb + n, None].to_broadcast(
                                [128, n, 128]),
                            in1=iota_t[:, None, :].to_broadcast([128, n, 128]),
                            op=mybir.AluOpType.is_equal,
                        )
                        off += n

                    aggr = psum.tile([F, 128], f32, tag="aggr", bufs=2)
                    j = 0
                    for q in range(NQ):
                        n = int(nchq[t, q])
                        lc = int(colbase[t, q]) - gcs
                        for k in range(n):
                            nc.tensor.matmul(aggr[:], gbuf[:, lc + k, :],
                                             oh[:, j, :],
                                             start=(j == 0),
                                             stop=(j == ncht - 1))
                            j += 1

                    xT_sl = xT_res[:, ts:ts + tw]
                    h = wpool.tile([F, 128], f32, tag="h")
                    nc.vector.tensor_scalar(
                        out=h[:, :tw], in0=xT_sl,
                        scalar1=eps_t[:, layer:layer + 1],
                        scalar2=None, op0=mybir.AluOpType.mult)
                    nc.vector.tensor_tensor(
                        out=h[:, :tw], in0=h[:, :tw], in1=aggr[:, :tw],
                        op=mybir.AluOpType.add)

                    p1 = psum.tile([F, 128], f32, tag="p1", bufs=1)
                    nc.tensor.matmul(p1[:, :tw],
                                     w1_t[:, layer * F:(layer + 1) * F],
                                     h[:, :tw], start=True, stop=True)
                    r1 = wpool.tile([F, 128], f32, tag="r1")
                    nc.scalar.activation(r1[:, :tw], p1[:, :tw],
                                         mybir.ActivationFunctionType.Relu,
                                         bias=b1_t[:, layer:layer + 1])

                    p2 = psum.tile([F, 128], f32, tag="p2", bufs=1)
                    nc.tensor.matmul(p2[:, :tw],
                                     w2_t[:, layer * F:(layer + 1) * F],
                                     r1[:, :tw], start=True, stop=True)

                    o = wpool.tile([F, 128], f32, tag="o")
                    if layer > 0:
                        nc.vector.tensor_tensor(out=o[:, :tw], in0=p2[:, :tw],
                                                in1=h[:, :tw],
                                                op=mybir.AluOpType.add)
                        nc.scalar.activation(o[:, :tw], o[:, :tw],
                                             mybir.ActivationFunctionType.Relu,
                                             bias=b2_t[:, layer:layer + 1])
                    else:
                        nc.scalar.activation(o[:, :tw], p2[:, :tw],
                                             mybir.ActivationFunctionType.Relu,
                                             bias=b2_t[:, layer:layer + 1])
                    nc.vector.tensor_tensor(out=xT_sl, in0=o[:, :tw],
                                            in1=xT_sl,
                                            op=mybir.AluOpType.add)

                    pt = psum.tile([128, F], f32, tag="pt", bufs=2)
                    nc.tensor.transpose(out=pt[:tw, :],
                                        in_=xT_res[:, ts:ts + tw],
                                        identity=ident_t[:])
                    st = wpool.tile([128, F], b16, tag="st")
                    nc.vector.tensor_copy(st[:tw, :], pt[:tw, :])
                    if not last:
                        nc.sync.dma_start(newx[ts:ts + tw, :], st[:tw, :])
                    else:
                        sel = wpool.tile([128, NUM_GRAPHS], b16, tag="sel")
                        nc.vector.tensor_tensor(
                            out=sel[:],
                            in0=bl_t[:, t:t + 1].to_broadcast(
                                [128, NUM_GRAPHS]),
                            in1=iota_t[:, :NUM_GRAPHS],
                            op=mybir.AluOpType.is_equal,
                        )
                        nc.tensor.matmul(pool_ps[:], st[:], sel[:],
                                         start=(t == 0), stop=(t == NT - 1))

                    # fire this quarter's AllGather as soon as its tiles done
                    if not last:
                        for q in range(NQ):
                            if t == QT[q + 1] - 1:
                                nc.gpsimd.collective_compute(
                                    "AllGather", mybir.AluOpType.bypass,
                                    ins=[newx[QROW[q]:QROW[q + 1], :]],
                                    outs=[wset[q][:]], replica_groups=rg)

        # ---- mean pool + classifier ----
        pacc = wpool.tile([128, NUM_GRAPHS], f32, tag="pacc")
        nc.vector.tensor_tensor(out=pacc[:], in0=pool_ps[:], in1=inv_t[:],
                                op=mybir.AluOpType.mult)
        nc.sync.dma_start(pr_in[:], pacc[:])
        nc.gpsimd.collective_compute(
            "AllReduce", mybir.AluOpType.add,
            ins=[pr_in[:]], outs=[pr_out[:]], replica_groups=rg)
        pooled = wpool.tile([128, NUM_GRAPHS], f32, tag="pooled")
        nc.sync.dma_start(pooled[:], pr_out[:])

        pc1 = psum.tile([F, NUM_GRAPHS], f32, tag="aggr", bufs=2)
        nc.tensor.matmul(pc1[:], wc1_t[:], pooled[:], start=True, stop=True)
        rc1 = wpool.tile([F, NUM_GRAPHS], f32, tag="rc1")
        nc.scalar.activation(rc1[:], pc1[:], mybir.ActivationFunctionType.Relu,
                             bias=bc1_t[:])
        pc2 = psum.tile([NUM_CLASSES, NUM_GRAPHS], f32, tag="p1", bufs=1)
        nc.tensor.matmul(pc2[:], wc2_t[:], rc1[:], start=True, stop=True)
        lg = wpool.tile([NUM_CLASSES, NUM_GRAPHS], f32, tag="lg")
        nc.vector.tensor_scalar(out=lg[:], in0=pc2[:], scalar1=bc2_t[:],
                                scalar2=None, op0=mybir.AluOpType.add)
        nc.sync.dma_start(out_t[:], lg[:])
        ctx.close()

    nc.compile()
    return nc


def _get_module(nchq, colbase, call_start, call_cols):
    key = tuple(nchq.reshape(-1).tolist())
    if key not in _CACHE:
        _CACHE.clear()
        _CACHE[key] = _build(nchq, colbase, call_start, call_cols)
    return _CACHE[key]


def kernel(x, edge_index, batch, eps, W1, b1, W2, b2, Wc1, bc1, Wc2, bc2,
           _trace=False):
    from concourse.bass_utils import run_bass_kernel_spmd

    x = np.ascontiguousarray(np.asarray(x), dtype=np.float32)
    eps = np.asarray(eps, dtype=np.float32)
    W1 = np.asarray(W1, dtype=np.float32)
    b1 = np.asarray(b1, dtype=np.float32)
    W2 = np.asarray(W2, dtype=np.float32)
    b2 = np.asarray(b2, dtype=np.float32)

    (gidx, gdst, nchq, colbase, call_start, call_cols,
     blocal, inv) = _preprocess(np.asarray(edge_index), np.asarray(batch))
    nc = _get_module(nchq, colbase, call_start, call_cols)

    # quartered bf16 replicas of the input features
    xbf = x.astype(BF16)
    x0q = []
    for q in range(NQ):
        parts = [xbf[c * NPC + QROW[q]: c * NPC + QROW[q + 1]]
                 for c in range(N_CORES)]
        x0q.append(np.ascontiguousarray(np.concatenate(parts, axis=0)))

    L = NUM_LAYERS
    common = {
        "iota": np.ascontiguousarray(
            np.broadcast_to(np.arange(128, dtype=np.float32),
                            (128, 128))).astype(BF16),
        "ident": np.eye(128, dtype=np.float32),
        "w1cat": np.ascontiguousarray(np.concatenate(list(W1[:L]), axis=1)),
        "w2cat": np.ascontiguousarray(np.concatenate(list(W2[:L]), axis=1)),
        "b1t": np.ascontiguousarray(b1[:L].T),
        "b2t": np.ascontiguousarray(b2[:L].T),
        "epsb": np.ascontiguousarray(
            np.broadcast_to(1.0 + eps[:L], (F, L))),
        "wc1": np.ascontiguousarray(np.asarray(Wc1, np.float32)),
        "bc1c": np.ascontiguousarray(np.asarray(bc1, np.float32)[:, None]),
        "wc2": np.ascontiguousarray(np.asarray(Wc2, np.float32)),
        "bc2c": np.ascontiguousarray(np.asarray(bc2, np.float32)[:, None]),
        "invc": np.ascontiguousarray(np.broadcast_to(inv, (128, NUM_GRAPHS))),
    }
    for q in range(NQ):
        common[f"x0q{q}"] = x0q[q]

    in_maps = []
    for c in range(N_CORES):
        m = dict(common)
        m["xT_own"] = np.ascontiguousarray(x[c * NPC:(c + 1) * NPC].T)
        m["gidx"] = gidx[c]
        m["gdst"] = gdst[c].astype(BF16)
        m["blocal"] = blocal[c].astype(BF16)
        in_maps.append(m)

    res = run_bass_kernel_spmd(nc, in_maps, core_ids=list(range(N_CORES)),
                               trace=_trace)
    out = np.ascontiguousarray(res.results[0]["logits_t"].T)
    if _trace:
        kernel._last_result = res
    return out
